# revision 21
# baseline (speedup 1.0000x reference)
"""Trainium2 Bass kernel for nn_DCT_Forward_Model (JPEG-style DCT quantize/dequantize).

Math: the reference output equals the approx_dct forward path:
  B = img - 128 (per 8x8 block), t22 = (X @ B @ X^T)/sf^2 with X = fl32(D*65000),
  q = round(t22/Q50 + 1e-6), deq = Q50*q, t2 = (X^T @ deq @ X)/sf^2, out = round(t2)+128.
(The grad path g cancels: out = g + stopgrad(a - g) == a up to fp noise.)

Kernel formulation (per NeuronCore, pure data parallel over images):
  - tiles of TI=125 images, GROUP=4 tiles per group (N=500 moving cols)
  - PE transposes 8x [125,128] -> vec-layout chunks V_q [128, 125]
    (chunk q holds image rows 4q..4q+3, vec index p = (r-4q)*32 + c).
    Transposes run in FLOAT32R via bitcast (1.5 cyc/row vs fp32's 2.0;
    the rounding is identical to what the f32r forward matmul does anyway)
    and all 8 land in one fused PSUM tile [128, 8, 128] (bank-aligned 512B
    regions) -> ONE PSUM->SBUF copy per sub-tile.
  - forward 2D DCT as fused Kronecker matmuls in FLOAT32R (1 cyc/row at
    N>=256): t22 chunk p' accumulates 2 matmuls with constant [128,128]
    weight slices. 1/Q50 is folded into W1 (per-out-partition scale), so
    PSUM holds t22/Q directly.
  - forward chunks are processed in same-parity pairs into a [128, 2, 512]
    PSUM tile so each quantize op covers 2 chunks: ACT u = Copy(t22q+MAGIC)
    snaps to integer (RNE), DVE subtracts MAGIC (odd parity immediate; even
    parity per-partition AP folding the -128 DC correction) -> bf16 q.
  - inverse DCT (bf16) with the DATA as stationary operand: out[img, pix]
    block jb accumulates 2 matmuls lhsT=q_chunk [128,128], rhs=W2 [128,256]
    with Q50 dequant, 1/sf^2 and the int8 0.5 output scale folded in ->
    natural [img, pixel] raster layout via W2 column ordering.
  - output stored int8 as round(t2/2) via ACT bias=MAGIC then DVE subtract
    (host unpacks 2*y+128; drops the LSB, ~0.005 rel err)
  - DMA: contiguous per-image transfers; loads/stores ride configurable
    rings (LOAD_RINGS round-robin) to spread across HW DGE queues.
"""

import os
import sys
import numpy as np
from contextlib import ExitStack

if "/opt/trn_rl_repo" not in sys.path and os.path.isdir("/opt/trn_rl_repo"):
    sys.path.insert(0, "/opt/trn_rl_repo")

N_CORES = 8
SIZE = 20000
PER_CORE = SIZE // N_CORES  # 2500
TI = 125                    # images per tile
NT = PER_CORE // TI         # 20 tiles per core
GROUP = 4                   # tiles per forward-stationary group (N=500 >= 256 for f32r)
PAIRS = ((0, 2), (4, 6), (1, 3), (5, 7))  # same-parity forward chunk pairs
TR_F32R = False             # f32r transposes rejected: BIR verifier requires
                            # f32r matmul inputs to come from a rounding
                            # producer (bitcast doesn't count)
STAGES = ("tr", "fwd", "inv")   # ablation control (bench only)
IOP_BUFS = 6                # io pool slots per tag
VP_BUFS = 3                 # v pool slots per tag
PT22_BUFS = 1               # PSUM bufs for paired t22 ([128,2,512] = 2 banks each)
POUT_BUFS = 2               # out PSUM bufs (2 banks each)
PTP_BUFS = 1                # fused transpose PSUM bufs (2 banks each)
LOAD_RINGS = ("gpsimd",)    # round-robin DMA queues for input loads
                            # (SWDGE measures ~170GB/s vs sync HWDGE ~120)
DMA_BIG = False             # one rearranged dma_start per group instead of 4
QUAD = True                 # images 4p..4p+3 on partition p: 16KB-contiguous
                            # load descriptors, 4KB store descriptors, one
                            # dma_start each per group (pure image permute)
XP_BUFS = 3                 # xin group buffers (16KB/partition each, QUAD)
YOUT_BUFS = 4               # yout tile buffers
YIG_BUFS = 2                # yig group store buffers (QUAD)
QT_BUFS = 2                 # qt buffers
U_BUFS = 3                  # u buffers
STORE_RINGS = ("gpsimd",)   # round-robin DMA queues for output stores
A_ENGS = ("vector", "vector", "vector", "vector")  # per-sub V-copy engine
B1_ENG = "scalar"           # quantize magic-add engine (reads PSUM)
B2_ENG = "vector"           # quantize subtract engine
C1_ENG = "scalar"           # inverse magic-add engine (reads PSUM)
C2_ENG = "vector"           # inverse subtract engine
MAGIC = 12582912.0          # 1.5 * 2^23: fp32 add snaps to integer (RNE)

_Q50 = np.array(
    [[16, 11, 10, 16, 24, 40, 51, 61], [12, 12, 14, 19, 26, 58, 60, 55],
     [14, 13, 16, 24, 40, 57, 69, 56], [14, 17, 22, 29, 51, 87, 80, 62],
     [18, 22, 37, 56, 68, 109, 103, 77], [24, 35, 55, 64, 81, 104, 113, 92],
     [49, 64, 78, 87, 103, 121, 120, 101], [72, 92, 95, 98, 112, 100, 103, 99]],
    dtype=np.float32)


def _dct_mat8():
    k = np.arange(8)[:, None]
    n = np.arange(8)[None, :]
    D = np.cos(np.pi * k * (2 * n + 1) / 16.0)
    D[0] *= np.sqrt(1.0 / 8.0)
    D[1:] *= np.sqrt(2.0 / 8.0)
    return D.astype(np.float32)


def _build_constants(weight=None, wf=65000.0):
    SF = np.float64(wf)
    if weight is None:
        Xbase = _dct_mat8()
    else:
        Xbase = np.asarray(weight, dtype=np.float32).reshape(8, 8)
    X = (Xbase * np.float32(wf)).astype(np.float32)
    X64 = X.astype(np.float64)
    Q64 = _Q50.astype(np.float64)

    ii_, kk = np.arange(4), np.arange(32)
    jj_, cc = np.arange(4), np.arange(32)
    blkmask = (cc[:, None] // 8 == kk[None, :] // 8)  # [c, k]

    # W1[(jj,c), m=(p_*2+qi), (ii,k)] = X[i%8,j%8]*X[k%8,c%8]/(sf^2*Q50[i%8,k%8])
    W1 = np.zeros((128, 16, 128), dtype=np.float64)
    for p_ in range(8):
        jb = p_ // 2
        for qi in range(2):
            q = 2 * jb + qi
            m = p_ * 2 + qi
            i8 = (4 * p_ + ii_) % 8
            j8 = (4 * q + jj_) % 8
            a = X64[i8[None, :], j8[:, None]]            # [jj, ii]
            b = np.where(blkmask, X64[kk[None, :] % 8, cc[:, None] % 8], 0.0)  # [c,k]
            invq = 1.0 / Q64[i8[:, None], kk[None, :] % 8]  # [ii, k]
            W1[:, m, :] = (np.einsum('ji,ck,ik->jcik', a, b, invq)
                           / (SF * SF)).reshape(128, 128)

    # W2[(jj,c), q, m2=i8*32+k] = X[j%8,i%8]*X[c%8,k%8]*Q50[j%8,c%8]*0.5/sf^2
    # (rows = coef chunk-q partitions (jj,c); cols = output pixel (i8,k);
    #  dequant Q50 indexed by the coef, 0.5 = int8 output halving fold)
    W2 = np.zeros((128, 8, 256), dtype=np.float64)
    i8_ = np.arange(8)
    for q in range(8):
        j8 = (4 * q + jj_) % 8
        a = X64[j8[:, None], i8_[None, :]]               # [jj, i8]
        b = np.where(blkmask, X64[cc[:, None] % 8, kk[None, :] % 8], 0.0)  # [c,k]
        qf = Q64[j8[:, None], cc[None, :] % 8]           # [jj, c]
        W2[:, q, :] = (np.einsum('ji,ck,jc->jcik', a, b, qf)
                       * 0.5 / (SF * SF)).reshape(128, 256)

    # quantize subtract vectors: partition p=(ii,k); even chunks i8=ii,
    # odd chunks i8=ii+4. csub = MAGIC - rint(-128 DC fold / Q)
    Sx = X64.sum(axis=1)
    csub = np.zeros((128, 2), dtype=np.float32)
    for par in range(2):
        for ii in range(4):
            for k in range(32):
                p = ii * 32 + k
                i8 = ii + 4 * par
                qv = Q64[i8, k % 8]
                c = -128.0 * Sx[i8] * Sx[k % 8] / (SF * SF) / qv
                csub[p, par] = np.float32(MAGIC - np.rint(c))
    return (np.ascontiguousarray(W1.astype(np.float32).reshape(128, 16 * 128)),
            np.ascontiguousarray(W2.astype(np.float32).reshape(128, 8 * 256)),
            csub)


def _build_nc(reps=1):
    import concourse.bacc as bacc
    import concourse.mybir as mybir
    from concourse import tile
    from concourse import bass
    from concourse.masks import make_identity

    f32 = mybir.dt.float32
    bf16 = mybir.dt.bfloat16
    f32r = mybir.dt.float32r

    nc = bacc.Bacc("TRN2", target_bir_lowering=False, debug=False,
                   num_devices=N_CORES)
    x = nc.dram_tensor("x", [PER_CORE, 1024], f32, kind="ExternalInput")
    w1 = nc.dram_tensor("w1", [128, 2048], f32, kind="ExternalInput")
    w2 = nc.dram_tensor("w2", [128, 2048], bf16, kind="ExternalInput")
    qv = nc.dram_tensor("qv", [128, 2], f32, kind="ExternalInput")  # csubE, csubO
    y = nc.dram_tensor("y", [PER_CORE, 1024], mybir.dt.int8, kind="ExternalOutput")

    with tile.TileContext(nc) as tc, ExitStack() as ctx:
        consts = ctx.enter_context(tc.tile_pool(name="consts", bufs=1))
        pools = {
            "xin": ctx.enter_context(tc.tile_pool(name="xp", bufs=XP_BUFS)),
            "V": ctx.enter_context(tc.tile_pool(name="vp", bufs=VP_BUFS)),
            "qt": ctx.enter_context(tc.tile_pool(name="qp", bufs=QT_BUFS)),
            "u": ctx.enter_context(tc.tile_pool(name="up", bufs=U_BUFS)),
            "yout": ctx.enter_context(tc.tile_pool(name="yo", bufs=YOUT_BUFS)),
            "yig": ctx.enter_context(tc.tile_pool(name="yg", bufs=YIG_BUFS)),
            "yi": ctx.enter_context(tc.tile_pool(name="yp", bufs=IOP_BUFS)),
        }
        ptp = ctx.enter_context(tc.tile_pool(name="ptp", bufs=PTP_BUFS, space=bass.MemorySpace.PSUM))
        pt22 = ctx.enter_context(tc.tile_pool(name="pt22", bufs=PT22_BUFS, space=bass.MemorySpace.PSUM))
        pout = ctx.enter_context(tc.tile_pool(name="pout", bufs=POUT_BUFS, space=bass.MemorySpace.PSUM))

        w1f_sb = consts.tile([128, 2048], f32)
        w1_sb = consts.tile([128, 2048], f32r)
        w2_sb = consts.tile([128, 2048], bf16)
        qv_sb = consts.tile([128, 2], f32)
        identf = consts.tile([128, 128], f32)
        ident = consts.tile([128, 128], f32r if TR_F32R else f32)
        nc.sync.dma_start(w1f_sb[:], w1[:])
        nc.sync.dma_start(w2_sb[:], w2[:])
        nc.sync.dma_start(qv_sb[:], qv[:])
        make_identity(nc, identf[:])
        nc.vector.tensor_copy(ident[:], identf[:])
        nc.vector.tensor_copy(w1_sb[:], w1f_sb[:])

        def body():
            for g in range(NT // GROUP):
                _group_body(nc, tc, mybir, g, x, y, w1_sb, w2_sb, qv_sb, ident,
                            pools, ptp, pt22, pout)

        if reps == 1:
            body()
        else:
            with tc.For_i(0, reps, 1):
                body()

    nc.compile()
    return nc


def _group_body(nc, tc, mybir, g, x, y, w1_sb, w2_sb, qv_sb, ident,
                pools, ptp, pt22, pout):
    """Process GROUP tiles (GROUP*TI images): shared-stationary forward MMs."""
    f32 = mybir.dt.float32
    bf16 = mybir.dt.bfloat16
    f32r = mybir.dt.float32r
    Copy = mybir.ActivationFunctionType.Copy
    base = g * GROUP * TI
    N = GROUP * TI

    def eng(name):
        return {"sync": nc.sync, "scalar": nc.scalar, "vector": nc.vector,
                "tensor": nc.tensor, "gpsimd": nc.gpsimd}[name]

    tdt = f32r if TR_F32R else f32

    # ---- load + transpose all GROUP tiles into V [128, 8, GROUP, TI] ----
    # (V is f32r: the PSUM->SBUF copy is the f32r rounding producer)
    V = pools["V"].tile([128, 8, GROUP, TI], f32r, tag="V")
    xins = []
    yig = None
    if QUAD:
        # partition p holds images base+4p..base+4p+3: load descriptors are
        # 16KB contiguous HBM runs, one dma_start per group
        xing = pools["xin"].tile([TI, GROUP, 1024], f32, tag="xin")
        ld = LOAD_RINGS[g % len(LOAD_RINGS)]
        eng(ld).dma_start(
            xing[:].rearrange("p s f -> p (s f)"),
            x[base:base + GROUP * TI, :].rearrange("(p s) f -> p (s f)", s=GROUP))
        xins = [xing[:, sub, :] for sub in range(GROUP)]
        yig = pools["yig"].tile([TI, GROUP, 1024], mybir.dt.int8, tag="yig")
    elif DMA_BIG:
        xing = pools["xin"].tile([TI, GROUP, 1024], f32, tag="xin")
        ld = LOAD_RINGS[g % len(LOAD_RINGS)]
        eng(ld).dma_start(
            xing[:],
            x[base:base + GROUP * TI, :].rearrange("(s p) f -> p s f", p=TI))
        xins = [xing[:, sub, :] for sub in range(GROUP)]
    else:
        for sub in range(GROUP):
            xin = pools["xin"].tile([TI, 1024], f32, tag="xin")
            xins.append(xin[:])
            ld = LOAD_RINGS[(g * GROUP + sub) % len(LOAD_RINGS)]
            eng(ld).dma_start(xin[:], x[base + sub * TI:base + (sub + 1) * TI, :])

    def store(sub, src):
        st = STORE_RINGS[(g * GROUP + sub) % len(STORE_RINGS)]
        eng(st).dma_start(y[base + sub * TI:base + (sub + 1) * TI, :], src)

    def store_quad_flush():
        st = STORE_RINGS[g % len(STORE_RINGS)]
        eng(st).dma_start(
            y[base:base + GROUP * TI, :].rearrange("(p s) f -> p (s f)", s=GROUP),
            yig[:].rearrange("p s f -> p (s f)"))

    if "tr" in STAGES:
        for sub in range(GROUP):
            xi = xins[sub]
            if TR_F32R:
                xi = xi.bitcast(f32r)
            tp = ptp.tile([128, 8, 128], tdt, tag="tp")  # 8 bank-aligned chunks
            for q in range(8):
                nc.tensor.transpose(
                    tp[:, q, 0:TI],
                    xi[:, q * 128:(q + 1) * 128],
                    ident[:TI, :TI])
            eng(A_ENGS[sub]).tensor_copy(V[:, :, sub, :], tp[:, :, 0:TI])
    def bogus_stores():
        # ablation: bogus passthrough output (bitcast to match y dtype+volume)
        if QUAD:
            st = STORE_RINGS[g % len(STORE_RINGS)]
            eng(st).dma_start(
                y[base:base + GROUP * TI, :].rearrange("(p s) f -> p (s f)", s=GROUP),
                xing[:].rearrange("p s f -> p (s f)").bitcast(mybir.dt.int8)[:, 0:GROUP * 1024])
        else:
            for sub in range(GROUP):
                store(sub, xins[sub][:, 0:256].bitcast(mybir.dt.int8))

    if "tr" not in STAGES or "fwd" not in STAGES:
        bogus_stores()
        return

    # ---- forward + quantize, same-parity chunk pairs ----
    # qt pair-major: [128, pair(4), idx(2), GROUP, 128]; chunk p_ at POS[p_]
    POS = {}
    for pi, pr in enumerate(PAIRS):
        for idx, p_ in enumerate(pr):
            POS[p_] = (pi, idx)
    qt = pools["qt"].tile([128, 4, 2, GROUP, 128], bf16, tag="qt")
    for pi, pr in enumerate(PAIRS):
        par = pr[0] % 2
        t22 = pt22.tile([128, 2, 512], f32, tag="t22")  # 2 bank-aligned chunks
        for idx, p_ in enumerate(pr):
            jb = p_ // 2
            for qi in range(2):
                m = p_ * 2 + qi
                nc.tensor.matmul(
                    t22[:, idx, 0:N],
                    w1_sb[:, m * 128:(m + 1) * 128],
                    V[:, 2 * jb + qi, :, :],
                    start=(qi == 0), stop=(qi == 1))
        u = pools["u"].tile([128, 2, N], f32, tag="u")
        eng(B1_ENG).activation(u[:], t22[:, :, 0:N], Copy, bias=MAGIC, scale=1.0)
        qdst = qt[:, pi, :, :, 0:TI]
        uv = u[:].rearrange("p a (s t) -> p a s t", s=GROUP, t=TI)
        sub_v = qv_sb[:, par:par + 1] if par == 0 else MAGIC
        eng(B2_ENG).tensor_scalar_sub(qdst, uv, sub_v)

    if "inv" not in STAGES:
        bogus_stores()
        return

    # ---- inverse per tile: out[img, pix] block jb accumulates chunks 2jb, 2jb+1
    for sub in range(GROUP):
        outP = pout.tile([128, 1024], f32, tag="outP")
        for jb in range(4):
            for qi in range(2):
                q = 2 * jb + qi
                pi, idx = POS[q]
                nc.tensor.matmul(
                    outP[:, jb * 256:(jb + 1) * 256],
                    qt[:, pi, idx, sub, :],
                    w2_sb[:, q * 256:(q + 1) * 256],
                    start=(qi == 0), stop=(qi == 1))
        yout = pools["yout"].tile([TI, 1024], f32, tag="yout")
        eng(C1_ENG).activation(yout[:], outP[0:TI, :], Copy, bias=MAGIC, scale=1.0)
        if QUAD:
            eng(C2_ENG).tensor_scalar_sub(yig[:, sub, :], yout[:], MAGIC)
        else:
            yi = pools["yi"].tile([TI, 1024], mybir.dt.int8, tag="yi")
            eng(C2_ENG).tensor_scalar_sub(yi[:], yout[:], MAGIC)
            store(sub, yi[:])
    if QUAD:
        store_quad_flush()


_NC_CACHE = None
PROFILE = False       # test.py sets this to capture an NTFF trace
LAST_RESULT = None    # BassKernelResults of the last run (for exec_time_ns)


def make_in_maps(inputs):
    import ml_dtypes
    x = np.ascontiguousarray(np.asarray(inputs["input"], dtype=np.float32))
    S = x.shape[0]
    assert S == SIZE, f"expected {SIZE} images, got {S}"
    xf = x.reshape(N_CORES, PER_CORE, 1024)

    w = inputs.get("weight")
    wf = inputs.get("weight_factor")
    wfv = float(np.asarray(wf).reshape(-1)[0]) if wf is not None else 65000.0
    if w is not None:
        w = np.asarray(w, dtype=np.float32)
        assert w.shape[0] == 1, "kernel supports n_mult=1"
        w = w[0]
    W1, W2, csub = _build_constants(w, wfv)
    W2 = np.ascontiguousarray(W2.astype(ml_dtypes.bfloat16))
    return [
        {"x": np.ascontiguousarray(xf[c]), "w1": W1, "w2": W2, "qv": csub}
        for c in range(N_CORES)
    ]


def kernel(**inputs) -> np.ndarray:
    global _NC_CACHE, LAST_RESULT
    from concourse.bass_utils import run_bass_kernel_spmd

    in_maps = make_in_maps(inputs)
    if _NC_CACHE is None:
        _NC_CACHE = _build_nc()
    nc = _NC_CACHE
    res = run_bass_kernel_spmd(nc, in_maps, core_ids=list(range(N_CORES)),
                               trace=PROFILE)
    LAST_RESULT = res
    out = np.stack([res.results[c]["y"] for c in range(N_CORES)], axis=0)
    out = out.reshape(1, 1, SIZE, 32, 32).astype(np.float32)
    out = out * 2.0 + 128.0  # device stored round(t2/2)
    return out


if __name__ == "__main__":
    rng = np.random.default_rng(0)
    x = (rng.random((SIZE, 1, 32, 32)) * 255).astype(np.float32)
    y = kernel(input=x)
    print("kernel ran, out shape", y.shape, y.dtype)


# revision 22
# speedup vs baseline: 1.0098x; 1.0098x over previous
"""Trainium2 Bass kernel for nn_DCT_Forward_Model (JPEG-style DCT quantize/dequantize).

Math: the reference output equals the approx_dct forward path:
  B = img - 128 (per 8x8 block), t22 = (X @ B @ X^T)/sf^2 with X = fl32(D*65000),
  q = round(t22/Q50 + 1e-6), deq = Q50*q, t2 = (X^T @ deq @ X)/sf^2, out = round(t2)+128.
(The grad path g cancels: out = g + stopgrad(a - g) == a up to fp noise.)

Kernel formulation (per NeuronCore, pure data parallel over images):
  - tiles of TI=125 images, GROUP=4 tiles per group (N=500 moving cols)
  - PE transposes 8x [125,128] -> vec-layout chunks V_q [128, 125]
    (chunk q holds image rows 4q..4q+3, vec index p = (r-4q)*32 + c).
    Transposes run in FLOAT32R via bitcast (1.5 cyc/row vs fp32's 2.0;
    the rounding is identical to what the f32r forward matmul does anyway)
    and all 8 land in one fused PSUM tile [128, 8, 128] (bank-aligned 512B
    regions) -> ONE PSUM->SBUF copy per sub-tile.
  - forward 2D DCT as fused Kronecker matmuls in FLOAT32R (1 cyc/row at
    N>=256): t22 chunk p' accumulates 2 matmuls with constant [128,128]
    weight slices. 1/Q50 is folded into W1 (per-out-partition scale), so
    PSUM holds t22/Q directly.
  - forward chunks are processed in same-parity pairs into a [128, 2, 512]
    PSUM tile so each quantize op covers 2 chunks: ACT u = Copy(t22q+MAGIC)
    snaps to integer (RNE), DVE subtracts MAGIC (odd parity immediate; even
    parity per-partition AP folding the -128 DC correction) -> bf16 q.
  - inverse DCT (bf16) with the DATA as stationary operand: out[img, pix]
    block jb accumulates 2 matmuls lhsT=q_chunk [128,128], rhs=W2 [128,256]
    with Q50 dequant, 1/sf^2 and the int8 0.5 output scale folded in ->
    natural [img, pixel] raster layout via W2 column ordering.
  - output stored int8 as round(t2/2) via ACT bias=MAGIC then DVE subtract
    (host unpacks 2*y+128; drops the LSB, ~0.005 rel err)
  - DMA: contiguous per-image transfers; loads/stores ride configurable
    rings (LOAD_RINGS round-robin) to spread across HW DGE queues.
"""

import os
import sys
import numpy as np
from contextlib import ExitStack

if "/opt/trn_rl_repo" not in sys.path and os.path.isdir("/opt/trn_rl_repo"):
    sys.path.insert(0, "/opt/trn_rl_repo")

N_CORES = 8
SIZE = 20000
PER_CORE = SIZE // N_CORES  # 2500
TI = 125                    # images per tile
NT = PER_CORE // TI         # 20 tiles per core
GROUP = 4                   # tiles per forward-stationary group (N=500 >= 256 for f32r)
PAIRS = ((0, 2), (4, 6), (1, 3), (5, 7))  # same-parity forward chunk pairs
TR_F32R = False             # f32r transposes rejected: BIR verifier requires
                            # f32r matmul inputs to come from a rounding
                            # producer (bitcast doesn't count)
STAGES = ("tr", "fwd", "inv")   # ablation control (bench only)
IOP_BUFS = 6                # io pool slots per tag
VP_BUFS = 3                 # v pool slots per tag
PT22_BUFS = 1               # PSUM bufs for paired t22 ([128,2,512] = 2 banks each)
POUT_BUFS = 2               # out PSUM bufs (2 banks each)
PTP_BUFS = 1                # fused transpose PSUM bufs (2 banks each)
LOAD_RINGS = ("gpsimd",)    # round-robin DMA queues for input loads
                            # (SWDGE measures ~170GB/s vs sync HWDGE ~120)
DMA_BIG = False             # one rearranged dma_start per group instead of 4
QUAD = True                 # images 4p..4p+3 on partition p: 16KB-contiguous
                            # load descriptors, 4KB store descriptors, one
                            # dma_start each per group (pure image permute)
XP_BUFS = 3                 # xin group buffers (16KB/partition each, QUAD)
YOUT_BUFS = 4               # yout tile buffers
YIG_BUFS = 2                # yig group store buffers (QUAD)
QT_BUFS = 2                 # qt buffers
U_BUFS = 3                  # u buffers
STORE_RINGS = ("sync",)     # stores on sync HWDGE: separate FIFO from
                            # the SWDGE load queue (same-FIFO stores make
                            # next group's load wait on this group's compute)
A_ENGS = ("vector", "vector", "vector", "vector")  # per-sub V-copy engine
B1_ENG = "scalar"           # quantize magic-add engine (reads PSUM)
B2_ENG = "vector"           # quantize subtract engine
C1_ENG = "scalar"           # inverse magic-add engine (reads PSUM)
C2_ENG = "vector"           # inverse subtract engine
MAGIC = 12582912.0          # 1.5 * 2^23: fp32 add snaps to integer (RNE)

_Q50 = np.array(
    [[16, 11, 10, 16, 24, 40, 51, 61], [12, 12, 14, 19, 26, 58, 60, 55],
     [14, 13, 16, 24, 40, 57, 69, 56], [14, 17, 22, 29, 51, 87, 80, 62],
     [18, 22, 37, 56, 68, 109, 103, 77], [24, 35, 55, 64, 81, 104, 113, 92],
     [49, 64, 78, 87, 103, 121, 120, 101], [72, 92, 95, 98, 112, 100, 103, 99]],
    dtype=np.float32)


def _dct_mat8():
    k = np.arange(8)[:, None]
    n = np.arange(8)[None, :]
    D = np.cos(np.pi * k * (2 * n + 1) / 16.0)
    D[0] *= np.sqrt(1.0 / 8.0)
    D[1:] *= np.sqrt(2.0 / 8.0)
    return D.astype(np.float32)


def _build_constants(weight=None, wf=65000.0):
    SF = np.float64(wf)
    if weight is None:
        Xbase = _dct_mat8()
    else:
        Xbase = np.asarray(weight, dtype=np.float32).reshape(8, 8)
    X = (Xbase * np.float32(wf)).astype(np.float32)
    X64 = X.astype(np.float64)
    Q64 = _Q50.astype(np.float64)

    ii_, kk = np.arange(4), np.arange(32)
    jj_, cc = np.arange(4), np.arange(32)
    blkmask = (cc[:, None] // 8 == kk[None, :] // 8)  # [c, k]

    # W1[(jj,c), m=(p_*2+qi), (ii,k)] = X[i%8,j%8]*X[k%8,c%8]/(sf^2*Q50[i%8,k%8])
    W1 = np.zeros((128, 16, 128), dtype=np.float64)
    for p_ in range(8):
        jb = p_ // 2
        for qi in range(2):
            q = 2 * jb + qi
            m = p_ * 2 + qi
            i8 = (4 * p_ + ii_) % 8
            j8 = (4 * q + jj_) % 8
            a = X64[i8[None, :], j8[:, None]]            # [jj, ii]
            b = np.where(blkmask, X64[kk[None, :] % 8, cc[:, None] % 8], 0.0)  # [c,k]
            invq = 1.0 / Q64[i8[:, None], kk[None, :] % 8]  # [ii, k]
            W1[:, m, :] = (np.einsum('ji,ck,ik->jcik', a, b, invq)
                           / (SF * SF)).reshape(128, 128)

    # W2[(jj,c), q, m2=i8*32+k] = X[j%8,i%8]*X[c%8,k%8]*Q50[j%8,c%8]*0.5/sf^2
    # (rows = coef chunk-q partitions (jj,c); cols = output pixel (i8,k);
    #  dequant Q50 indexed by the coef, 0.5 = int8 output halving fold)
    W2 = np.zeros((128, 8, 256), dtype=np.float64)
    i8_ = np.arange(8)
    for q in range(8):
        j8 = (4 * q + jj_) % 8
        a = X64[j8[:, None], i8_[None, :]]               # [jj, i8]
        b = np.where(blkmask, X64[cc[:, None] % 8, kk[None, :] % 8], 0.0)  # [c,k]
        qf = Q64[j8[:, None], cc[None, :] % 8]           # [jj, c]
        W2[:, q, :] = (np.einsum('ji,ck,jc->jcik', a, b, qf)
                       * 0.5 / (SF * SF)).reshape(128, 256)

    # quantize subtract vectors: partition p=(ii,k); even chunks i8=ii,
    # odd chunks i8=ii+4. csub = MAGIC - rint(-128 DC fold / Q)
    Sx = X64.sum(axis=1)
    csub = np.zeros((128, 2), dtype=np.float32)
    for par in range(2):
        for ii in range(4):
            for k in range(32):
                p = ii * 32 + k
                i8 = ii + 4 * par
                qv = Q64[i8, k % 8]
                c = -128.0 * Sx[i8] * Sx[k % 8] / (SF * SF) / qv
                csub[p, par] = np.float32(MAGIC - np.rint(c))
    return (np.ascontiguousarray(W1.astype(np.float32).reshape(128, 16 * 128)),
            np.ascontiguousarray(W2.astype(np.float32).reshape(128, 8 * 256)),
            csub)


def _build_nc(reps=1):
    import concourse.bacc as bacc
    import concourse.mybir as mybir
    from concourse import tile
    from concourse import bass
    from concourse.masks import make_identity

    f32 = mybir.dt.float32
    bf16 = mybir.dt.bfloat16
    f32r = mybir.dt.float32r

    nc = bacc.Bacc("TRN2", target_bir_lowering=False, debug=False,
                   num_devices=N_CORES)
    x = nc.dram_tensor("x", [PER_CORE, 1024], f32, kind="ExternalInput")
    w1 = nc.dram_tensor("w1", [128, 2048], f32, kind="ExternalInput")
    w2 = nc.dram_tensor("w2", [128, 2048], bf16, kind="ExternalInput")
    qv = nc.dram_tensor("qv", [128, 2], f32, kind="ExternalInput")  # csubE, csubO
    y = nc.dram_tensor("y", [PER_CORE, 1024], mybir.dt.int8, kind="ExternalOutput")

    with tile.TileContext(nc) as tc, ExitStack() as ctx:
        consts = ctx.enter_context(tc.tile_pool(name="consts", bufs=1))
        pools = {
            "xin": ctx.enter_context(tc.tile_pool(name="xp", bufs=XP_BUFS)),
            "V": ctx.enter_context(tc.tile_pool(name="vp", bufs=VP_BUFS)),
            "qt": ctx.enter_context(tc.tile_pool(name="qp", bufs=QT_BUFS)),
            "u": ctx.enter_context(tc.tile_pool(name="up", bufs=U_BUFS)),
            "yout": ctx.enter_context(tc.tile_pool(name="yo", bufs=YOUT_BUFS)),
            "yig": ctx.enter_context(tc.tile_pool(name="yg", bufs=YIG_BUFS)),
            "yi": ctx.enter_context(tc.tile_pool(name="yp", bufs=IOP_BUFS)),
        }
        ptp = ctx.enter_context(tc.tile_pool(name="ptp", bufs=PTP_BUFS, space=bass.MemorySpace.PSUM))
        pt22 = ctx.enter_context(tc.tile_pool(name="pt22", bufs=PT22_BUFS, space=bass.MemorySpace.PSUM))
        pout = ctx.enter_context(tc.tile_pool(name="pout", bufs=POUT_BUFS, space=bass.MemorySpace.PSUM))

        w1f_sb = consts.tile([128, 2048], f32)
        w1_sb = consts.tile([128, 2048], f32r)
        w2_sb = consts.tile([128, 2048], bf16)
        qv_sb = consts.tile([128, 2], f32)
        identf = consts.tile([128, 128], f32)
        ident = consts.tile([128, 128], f32r if TR_F32R else f32)
        nc.sync.dma_start(w1f_sb[:], w1[:])
        nc.sync.dma_start(w2_sb[:], w2[:])
        nc.sync.dma_start(qv_sb[:], qv[:])
        make_identity(nc, identf[:])
        nc.vector.tensor_copy(ident[:], identf[:])
        nc.vector.tensor_copy(w1_sb[:], w1f_sb[:])

        def body():
            for g in range(NT // GROUP):
                _group_body(nc, tc, mybir, g, x, y, w1_sb, w2_sb, qv_sb, ident,
                            pools, ptp, pt22, pout)

        if reps == 1:
            body()
        else:
            with tc.For_i(0, reps, 1):
                body()

    nc.compile()
    return nc


def _group_body(nc, tc, mybir, g, x, y, w1_sb, w2_sb, qv_sb, ident,
                pools, ptp, pt22, pout):
    """Process GROUP tiles (GROUP*TI images): shared-stationary forward MMs."""
    f32 = mybir.dt.float32
    bf16 = mybir.dt.bfloat16
    f32r = mybir.dt.float32r
    Copy = mybir.ActivationFunctionType.Copy
    base = g * GROUP * TI
    N = GROUP * TI

    def eng(name):
        return {"sync": nc.sync, "scalar": nc.scalar, "vector": nc.vector,
                "tensor": nc.tensor, "gpsimd": nc.gpsimd}[name]

    tdt = f32r if TR_F32R else f32

    # ---- load + transpose all GROUP tiles into V [128, 8, GROUP, TI] ----
    # (V is f32r: the PSUM->SBUF copy is the f32r rounding producer)
    V = pools["V"].tile([128, 8, GROUP, TI], f32r, tag="V")
    xins = []
    yig = None
    if QUAD:
        # partition p holds images base+4p..base+4p+3: load descriptors are
        # 16KB contiguous HBM runs, one dma_start per group
        xing = pools["xin"].tile([TI, GROUP, 1024], f32, tag="xin")
        ld = LOAD_RINGS[g % len(LOAD_RINGS)]
        eng(ld).dma_start(
            xing[:].rearrange("p s f -> p (s f)"),
            x[base:base + GROUP * TI, :].rearrange("(p s) f -> p (s f)", s=GROUP))
        xins = [xing[:, sub, :] for sub in range(GROUP)]
        yig = pools["yig"].tile([TI, GROUP, 1024], mybir.dt.int8, tag="yig")
    elif DMA_BIG:
        xing = pools["xin"].tile([TI, GROUP, 1024], f32, tag="xin")
        ld = LOAD_RINGS[g % len(LOAD_RINGS)]
        eng(ld).dma_start(
            xing[:],
            x[base:base + GROUP * TI, :].rearrange("(s p) f -> p s f", p=TI))
        xins = [xing[:, sub, :] for sub in range(GROUP)]
    else:
        for sub in range(GROUP):
            xin = pools["xin"].tile([TI, 1024], f32, tag="xin")
            xins.append(xin[:])
            ld = LOAD_RINGS[(g * GROUP + sub) % len(LOAD_RINGS)]
            eng(ld).dma_start(xin[:], x[base + sub * TI:base + (sub + 1) * TI, :])

    def store(sub, src):
        st = STORE_RINGS[(g * GROUP + sub) % len(STORE_RINGS)]
        eng(st).dma_start(y[base + sub * TI:base + (sub + 1) * TI, :], src)

    def store_quad_flush():
        st = STORE_RINGS[g % len(STORE_RINGS)]
        eng(st).dma_start(
            y[base:base + GROUP * TI, :].rearrange("(p s) f -> p (s f)", s=GROUP),
            yig[:].rearrange("p s f -> p (s f)"))

    if "tr" in STAGES:
        for sub in range(GROUP):
            xi = xins[sub]
            if TR_F32R:
                xi = xi.bitcast(f32r)
            tp = ptp.tile([128, 8, 128], tdt, tag="tp")  # 8 bank-aligned chunks
            for q in range(8):
                nc.tensor.transpose(
                    tp[:, q, 0:TI],
                    xi[:, q * 128:(q + 1) * 128],
                    ident[:TI, :TI])
            eng(A_ENGS[sub]).tensor_copy(V[:, :, sub, :], tp[:, :, 0:TI])
    def bogus_stores():
        # ablation: bogus passthrough output (bitcast to match y dtype+volume)
        if QUAD:
            st = STORE_RINGS[g % len(STORE_RINGS)]
            eng(st).dma_start(
                y[base:base + GROUP * TI, :].rearrange("(p s) f -> p (s f)", s=GROUP),
                xing[:].rearrange("p s f -> p (s f)").bitcast(mybir.dt.int8)[:, 0:GROUP * 1024])
        else:
            for sub in range(GROUP):
                store(sub, xins[sub][:, 0:256].bitcast(mybir.dt.int8))

    if "tr" not in STAGES or "fwd" not in STAGES:
        bogus_stores()
        return

    # ---- forward + quantize, same-parity chunk pairs ----
    # qt pair-major: [128, pair(4), idx(2), GROUP, 128]; chunk p_ at POS[p_]
    POS = {}
    for pi, pr in enumerate(PAIRS):
        for idx, p_ in enumerate(pr):
            POS[p_] = (pi, idx)
    qt = pools["qt"].tile([128, 4, 2, GROUP, 128], bf16, tag="qt")
    for pi, pr in enumerate(PAIRS):
        par = pr[0] % 2
        t22 = pt22.tile([128, 2, 512], f32, tag="t22")  # 2 bank-aligned chunks
        for idx, p_ in enumerate(pr):
            jb = p_ // 2
            for qi in range(2):
                m = p_ * 2 + qi
                nc.tensor.matmul(
                    t22[:, idx, 0:N],
                    w1_sb[:, m * 128:(m + 1) * 128],
                    V[:, 2 * jb + qi, :, :],
                    start=(qi == 0), stop=(qi == 1))
        u = pools["u"].tile([128, 2, N], f32, tag="u")
        eng(B1_ENG).activation(u[:], t22[:, :, 0:N], Copy, bias=MAGIC, scale=1.0)
        qdst = qt[:, pi, :, :, 0:TI]
        uv = u[:].rearrange("p a (s t) -> p a s t", s=GROUP, t=TI)
        sub_v = qv_sb[:, par:par + 1] if par == 0 else MAGIC
        eng(B2_ENG).tensor_scalar_sub(qdst, uv, sub_v)

    if "inv" not in STAGES:
        bogus_stores()
        return

    # ---- inverse per tile: out[img, pix] block jb accumulates chunks 2jb, 2jb+1
    for sub in range(GROUP):
        outP = pout.tile([128, 1024], f32, tag="outP")
        for jb in range(4):
            for qi in range(2):
                q = 2 * jb + qi
                pi, idx = POS[q]
                nc.tensor.matmul(
                    outP[:, jb * 256:(jb + 1) * 256],
                    qt[:, pi, idx, sub, :],
                    w2_sb[:, q * 256:(q + 1) * 256],
                    start=(qi == 0), stop=(qi == 1))
        yout = pools["yout"].tile([TI, 1024], f32, tag="yout")
        eng(C1_ENG).activation(yout[:], outP[0:TI, :], Copy, bias=MAGIC, scale=1.0)
        if QUAD:
            eng(C2_ENG).tensor_scalar_sub(yig[:, sub, :], yout[:], MAGIC)
        else:
            yi = pools["yi"].tile([TI, 1024], mybir.dt.int8, tag="yi")
            eng(C2_ENG).tensor_scalar_sub(yi[:], yout[:], MAGIC)
            store(sub, yi[:])
    if QUAD:
        store_quad_flush()


_NC_CACHE = None
PROFILE = False       # test.py sets this to capture an NTFF trace
LAST_RESULT = None    # BassKernelResults of the last run (for exec_time_ns)


def make_in_maps(inputs):
    import ml_dtypes
    x = np.ascontiguousarray(np.asarray(inputs["input"], dtype=np.float32))
    S = x.shape[0]
    assert S == SIZE, f"expected {SIZE} images, got {S}"
    xf = x.reshape(N_CORES, PER_CORE, 1024)

    w = inputs.get("weight")
    wf = inputs.get("weight_factor")
    wfv = float(np.asarray(wf).reshape(-1)[0]) if wf is not None else 65000.0
    if w is not None:
        w = np.asarray(w, dtype=np.float32)
        assert w.shape[0] == 1, "kernel supports n_mult=1"
        w = w[0]
    W1, W2, csub = _build_constants(w, wfv)
    W2 = np.ascontiguousarray(W2.astype(ml_dtypes.bfloat16))
    return [
        {"x": np.ascontiguousarray(xf[c]), "w1": W1, "w2": W2, "qv": csub}
        for c in range(N_CORES)
    ]


def kernel(**inputs) -> np.ndarray:
    global _NC_CACHE, LAST_RESULT
    from concourse.bass_utils import run_bass_kernel_spmd

    in_maps = make_in_maps(inputs)
    if _NC_CACHE is None:
        _NC_CACHE = _build_nc()
    nc = _NC_CACHE
    res = run_bass_kernel_spmd(nc, in_maps, core_ids=list(range(N_CORES)),
                               trace=PROFILE)
    LAST_RESULT = res
    out = np.stack([res.results[c]["y"] for c in range(N_CORES)], axis=0)
    out = out.reshape(1, 1, SIZE, 32, 32).astype(np.float32)
    out = out * 2.0 + 128.0  # device stored round(t2/2)
    return out


if __name__ == "__main__":
    rng = np.random.default_rng(0)
    x = (rng.random((SIZE, 1, 32, 32)) * 255).astype(np.float32)
    y = kernel(input=x)
    print("kernel ran, out shape", y.shape, y.dtype)


# revision 27
# speedup vs baseline: 1.5849x; 1.5695x over previous
"""Trainium2 Bass kernel for nn_DCT_Forward_Model (JPEG-style DCT quantize/dequantize).

Math: the reference output equals the approx_dct forward path:
  B = img - 128 (per 8x8 block), t22 = (X @ B @ X^T)/sf^2 with X = fl32(D*65000),
  q = round(t22/Q50 + 1e-6), deq = Q50*q, t2 = (X^T @ deq @ X)/sf^2, out = round(t2)+128.
(The grad path g cancels: out = g + stopgrad(a - g) == a up to fp noise.)

Kernel formulation (per NeuronCore, pure data parallel over images):
  - tiles of TI=125 images, GROUP=4 tiles per group (N=500 moving cols)
  - PE transposes 8x [125,128] -> vec-layout chunks V_q [128, 125]
    (chunk q holds image rows 4q..4q+3, vec index p = (r-4q)*32 + c).
    Transposes run in FLOAT32R via bitcast (1.5 cyc/row vs fp32's 2.0;
    the rounding is identical to what the f32r forward matmul does anyway)
    and all 8 land in one fused PSUM tile [128, 8, 128] (bank-aligned 512B
    regions) -> ONE PSUM->SBUF copy per sub-tile.
  - forward 2D DCT as fused Kronecker matmuls in FLOAT32R (1 cyc/row at
    N>=256): t22 chunk p' accumulates 2 matmuls with constant [128,128]
    weight slices. 1/Q50 is folded into W1 (per-out-partition scale), so
    PSUM holds t22/Q directly.
  - forward chunks are processed in same-parity pairs into a [128, 2, 512]
    PSUM tile so each quantize op covers 2 chunks: ACT u = Copy(t22q+MAGIC)
    snaps to integer (RNE), DVE subtracts MAGIC (odd parity immediate; even
    parity per-partition AP folding the -128 DC correction) -> bf16 q.
  - inverse DCT (bf16) with the DATA as stationary operand: out[img, pix]
    block jb accumulates 2 matmuls lhsT=q_chunk [128,128], rhs=W2 [128,256]
    with Q50 dequant, 1/sf^2 and the int8 0.5 output scale folded in ->
    natural [img, pixel] raster layout via W2 column ordering.
  - output stored int8 as round(t2/2) via ACT bias=MAGIC then DVE subtract
    (host unpacks 2*y+128; drops the LSB, ~0.005 rel err)
  - DMA: contiguous per-image transfers; loads/stores ride configurable
    rings (LOAD_RINGS round-robin) to spread across HW DGE queues.
"""

import os
import sys
import numpy as np
from contextlib import ExitStack

if "/opt/trn_rl_repo" not in sys.path and os.path.isdir("/opt/trn_rl_repo"):
    sys.path.insert(0, "/opt/trn_rl_repo")

N_CORES = 8
SIZE = 20000
PER_CORE = SIZE // N_CORES  # 2500
TI = 125                    # images per tile
NT = PER_CORE // TI         # 20 tiles per core
GROUP = 4                   # tiles per forward-stationary group (N=500 >= 256 for f32r)
PAIRS = ((0, 2), (4, 6), (1, 3), (5, 7))  # same-parity forward chunk pairs
TR_F32R = False             # f32r transposes rejected: BIR verifier requires
                            # f32r matmul inputs to come from a rounding
                            # producer (bitcast doesn't count)
STAGES = ("tr", "fwd", "inv")   # ablation control (bench only)
IOP_BUFS = 6                # io pool slots per tag
VP_BUFS = 3                 # v pool slots per tag
PT22_BUFS = 2               # PSUM bufs for per-chunk t22 ([128,512] = 1 bank)
POUT_BUFS = 2               # out PSUM bufs ([128,512] = 1 bank each)
PTP_BUFS = 2                # transpose PSUM bufs (tpA/tpB, 1 bank each)
LOAD_RINGS = ("gpsimd",)    # round-robin DMA queues for input loads
                            # (SWDGE measures ~170GB/s vs sync HWDGE ~120)
DMA_BIG = False             # one rearranged dma_start per group instead of 4
QUAD = False                # images 4p..4p+3 on partition p: 16KB-contiguous
                            # load descriptors, 4KB store descriptors, one
                            # dma_start each per group (pure image permute)
XP_BUFS = 6                 # xin tile buffers
YOUT_BUFS = 4               # yout tile buffers
YIG_BUFS = 2                # yig group store buffers (QUAD)
QT_BUFS = 2                 # qt buffers
U_BUFS = 3                  # u buffers
STORE_RINGS = ("sync",)     # stores on sync HWDGE: separate FIFO from
                            # the SWDGE load queue (same-FIFO stores make
                            # next group's load wait on this group's compute)
A_ENGS = ("vector", "vector", "vector", "vector")  # per-sub V-copy engine
B1_ENG = "scalar"           # quantize magic-add engine (reads PSUM)
B2_ENG = "vector"           # quantize subtract engine
C1_ENG = "scalar"           # inverse magic-add engine (reads PSUM)
C2_ENG = "vector"           # inverse subtract engine
MAGIC = 12582912.0          # 1.5 * 2^23: fp32 add snaps to integer (RNE)

_Q50 = np.array(
    [[16, 11, 10, 16, 24, 40, 51, 61], [12, 12, 14, 19, 26, 58, 60, 55],
     [14, 13, 16, 24, 40, 57, 69, 56], [14, 17, 22, 29, 51, 87, 80, 62],
     [18, 22, 37, 56, 68, 109, 103, 77], [24, 35, 55, 64, 81, 104, 113, 92],
     [49, 64, 78, 87, 103, 121, 120, 101], [72, 92, 95, 98, 112, 100, 103, 99]],
    dtype=np.float32)


def _dct_mat8():
    k = np.arange(8)[:, None]
    n = np.arange(8)[None, :]
    D = np.cos(np.pi * k * (2 * n + 1) / 16.0)
    D[0] *= np.sqrt(1.0 / 8.0)
    D[1:] *= np.sqrt(2.0 / 8.0)
    return D.astype(np.float32)


def _build_constants(weight=None, wf=65000.0):
    SF = np.float64(wf)
    if weight is None:
        Xbase = _dct_mat8()
    else:
        Xbase = np.asarray(weight, dtype=np.float32).reshape(8, 8)
    X = (Xbase * np.float32(wf)).astype(np.float32)
    X64 = X.astype(np.float64)
    Q64 = _Q50.astype(np.float64)

    ii_, kk = np.arange(4), np.arange(32)
    jj_, cc = np.arange(4), np.arange(32)
    blkmask = (cc[:, None] // 8 == kk[None, :] // 8)  # [c, k]

    # W1[(jj,c), m=(p_*2+qi), (ii,k)] = X[i%8,j%8]*X[k%8,c%8]/(sf^2*Q50[i%8,k%8])
    W1 = np.zeros((128, 16, 128), dtype=np.float64)
    for p_ in range(8):
        jb = p_ // 2
        for qi in range(2):
            q = 2 * jb + qi
            m = p_ * 2 + qi
            i8 = (4 * p_ + ii_) % 8
            j8 = (4 * q + jj_) % 8
            a = X64[i8[None, :], j8[:, None]]            # [jj, ii]
            b = np.where(blkmask, X64[kk[None, :] % 8, cc[:, None] % 8], 0.0)  # [c,k]
            invq = 1.0 / Q64[i8[:, None], kk[None, :] % 8]  # [ii, k]
            W1[:, m, :] = (np.einsum('ji,ck,ik->jcik', a, b, invq)
                           / (SF * SF)).reshape(128, 128)

    # W2[(jj,c), q, m2=i8*32+k] = X[j%8,i%8]*X[c%8,k%8]*Q50[j%8,c%8]*0.5/sf^2
    # (rows = coef chunk-q partitions (jj,c); cols = output pixel (i8,k);
    #  dequant Q50 indexed by the coef, 0.5 = int8 output halving fold)
    W2 = np.zeros((128, 8, 256), dtype=np.float64)
    i8_ = np.arange(8)
    for q in range(8):
        j8 = (4 * q + jj_) % 8
        a = X64[j8[:, None], i8_[None, :]]               # [jj, i8]
        b = np.where(blkmask, X64[cc[:, None] % 8, kk[None, :] % 8], 0.0)  # [c,k]
        qf = Q64[j8[:, None], cc[None, :] % 8]           # [jj, c]
        W2[:, q, :] = (np.einsum('ji,ck,jc->jcik', a, b, qf)
                       * 0.5 / (SF * SF)).reshape(128, 256)

    # quantize subtract vectors: partition p=(ii,k); even chunks i8=ii,
    # odd chunks i8=ii+4. csub = MAGIC - rint(-128 DC fold / Q)
    Sx = X64.sum(axis=1)
    csub = np.zeros((128, 2), dtype=np.float32)
    for par in range(2):
        for ii in range(4):
            for k in range(32):
                p = ii * 32 + k
                i8 = ii + 4 * par
                qv = Q64[i8, k % 8]
                c = -128.0 * Sx[i8] * Sx[k % 8] / (SF * SF) / qv
                csub[p, par] = np.float32(MAGIC - np.rint(c))
    return (np.ascontiguousarray(W1.astype(np.float32).reshape(128, 16 * 128)),
            np.ascontiguousarray(W2.astype(np.float32).reshape(128, 8 * 256)),
            csub)


def _build_nc(reps=1):
    import concourse.bacc as bacc
    import concourse.mybir as mybir
    from concourse import tile
    from concourse import bass
    from concourse.masks import make_identity

    f32 = mybir.dt.float32
    bf16 = mybir.dt.bfloat16
    f32r = mybir.dt.float32r

    nc = bacc.Bacc("TRN2", target_bir_lowering=False, debug=False,
                   num_devices=N_CORES)
    x = nc.dram_tensor("x", [PER_CORE, 1024], f32, kind="ExternalInput")
    w1 = nc.dram_tensor("w1", [128, 2048], f32, kind="ExternalInput")
    w2 = nc.dram_tensor("w2", [128, 2048], bf16, kind="ExternalInput")
    qv = nc.dram_tensor("qv", [128, 2], f32, kind="ExternalInput")  # csubE, csubO
    y = nc.dram_tensor("y", [PER_CORE, 1024], mybir.dt.int8, kind="ExternalOutput")

    with tile.TileContext(nc) as tc, ExitStack() as ctx:
        consts = ctx.enter_context(tc.tile_pool(name="consts", bufs=1))
        pools = {
            "xin": ctx.enter_context(tc.tile_pool(name="xp", bufs=XP_BUFS)),
            "V": ctx.enter_context(tc.tile_pool(name="vp", bufs=VP_BUFS)),
            "qt": ctx.enter_context(tc.tile_pool(name="qp", bufs=QT_BUFS)),
            "u": ctx.enter_context(tc.tile_pool(name="up", bufs=U_BUFS)),
            "yout": ctx.enter_context(tc.tile_pool(name="yo", bufs=YOUT_BUFS)),
            "yig": ctx.enter_context(tc.tile_pool(name="yg", bufs=YIG_BUFS)),
            "yi": ctx.enter_context(tc.tile_pool(name="yp", bufs=IOP_BUFS)),
        }
        ptp = ctx.enter_context(tc.tile_pool(name="ptp", bufs=PTP_BUFS, space=bass.MemorySpace.PSUM))
        pt22 = ctx.enter_context(tc.tile_pool(name="pt22", bufs=PT22_BUFS, space=bass.MemorySpace.PSUM))
        pout = ctx.enter_context(tc.tile_pool(name="pout", bufs=POUT_BUFS, space=bass.MemorySpace.PSUM))

        w1f_sb = consts.tile([128, 2048], f32)
        w1_sb = consts.tile([128, 2048], f32r)
        w2_sb = consts.tile([128, 2048], bf16)
        qv_sb = consts.tile([128, 2], f32)
        identf = consts.tile([128, 128], f32)
        ident = consts.tile([128, 128], f32r if TR_F32R else f32)
        nc.sync.dma_start(w1f_sb[:], w1[:])
        nc.sync.dma_start(w2_sb[:], w2[:])
        nc.sync.dma_start(qv_sb[:], qv[:])
        make_identity(nc, identf[:])
        nc.vector.tensor_copy(ident[:], identf[:])
        nc.vector.tensor_copy(w1_sb[:], w1f_sb[:])

        def body():
            for g in range(NT // GROUP):
                _group_body(nc, tc, mybir, g, x, y, w1_sb, w2_sb, qv_sb, ident,
                            pools, ptp, pt22, pout)

        if reps == 1:
            body()
        else:
            with tc.For_i(0, reps, 1):
                body()

    nc.compile()
    return nc


def _group_body(nc, tc, mybir, g, x, y, w1_sb, w2_sb, qv_sb, ident,
                pools, ptp, pt22, pout):
    """Process GROUP tiles (GROUP*TI images): shared-stationary forward MMs."""
    f32 = mybir.dt.float32
    bf16 = mybir.dt.bfloat16
    f32r = mybir.dt.float32r
    Copy = mybir.ActivationFunctionType.Copy
    base = g * GROUP * TI
    N = GROUP * TI

    def eng(name):
        return {"sync": nc.sync, "scalar": nc.scalar, "vector": nc.vector,
                "tensor": nc.tensor, "gpsimd": nc.gpsimd}[name]

    tdt = f32r if TR_F32R else f32

    # ---- load + transpose all GROUP tiles into V [128, 8, GROUP, TI] ----
    # (V is f32r: the PSUM->SBUF copy is the f32r rounding producer)
    V = pools["V"].tile([128, 8, GROUP, TI], f32r, tag="V")
    xins = []
    yig = None
    if QUAD:
        # partition p holds images base+4p..base+4p+3: load descriptors are
        # 16KB contiguous HBM runs, one dma_start per group
        xing = pools["xin"].tile([TI, GROUP, 1024], f32, tag="xin")
        ld = LOAD_RINGS[g % len(LOAD_RINGS)]
        eng(ld).dma_start(
            xing[:].rearrange("p s f -> p (s f)"),
            x[base:base + GROUP * TI, :].rearrange("(p s) f -> p (s f)", s=GROUP))
        xins = [xing[:, sub, :] for sub in range(GROUP)]
        yig = pools["yig"].tile([TI, GROUP, 1024], mybir.dt.int8, tag="yig")
    elif DMA_BIG:
        xing = pools["xin"].tile([TI, GROUP, 1024], f32, tag="xin")
        ld = LOAD_RINGS[g % len(LOAD_RINGS)]
        eng(ld).dma_start(
            xing[:],
            x[base:base + GROUP * TI, :].rearrange("(s p) f -> p s f", p=TI))
        xins = [xing[:, sub, :] for sub in range(GROUP)]
    else:
        for sub in range(GROUP):
            xin = pools["xin"].tile([TI, 1024], f32, tag="xin")
            xins.append(xin[:])
            ld = LOAD_RINGS[(g * GROUP + sub) % len(LOAD_RINGS)]
            eng(ld).dma_start(xin[:], x[base + sub * TI:base + (sub + 1) * TI, :])

    def store(sub, src):
        st = STORE_RINGS[(g * GROUP + sub) % len(STORE_RINGS)]
        eng(st).dma_start(y[base + sub * TI:base + (sub + 1) * TI, :], src)

    def store_quad_flush():
        st = STORE_RINGS[g % len(STORE_RINGS)]
        eng(st).dma_start(
            y[base:base + GROUP * TI, :].rearrange("(p s) f -> p (s f)", s=GROUP),
            yig[:].rearrange("p s f -> p (s f)"))

    if "tr" in STAGES:
        for sub in range(GROUP):
            xi = xins[sub]
            if TR_F32R:
                xi = xi.bitcast(f32r)
            for h in range(2):  # tpA / tpB halves: finer PSUM pipelining
                tp = ptp.tile([128, 4, 128], tdt, tag=f"tp{h}")
                for qq in range(4):
                    q = 4 * h + qq
                    nc.tensor.transpose(
                        tp[:, qq, 0:TI],
                        xi[:, q * 128:(q + 1) * 128],
                        ident[:TI, :TI])
                eng(A_ENGS[sub]).tensor_copy(V[:, 4 * h:4 * h + 4, sub, :],
                                             tp[:, :, 0:TI])
    def bogus_stores():
        # ablation: bogus passthrough output (bitcast to match y dtype+volume)
        if QUAD:
            st = STORE_RINGS[g % len(STORE_RINGS)]
            eng(st).dma_start(
                y[base:base + GROUP * TI, :].rearrange("(p s) f -> p (s f)", s=GROUP),
                xing[:].rearrange("p s f -> p (s f)").bitcast(mybir.dt.int8)[:, 0:GROUP * 1024])
        else:
            for sub in range(GROUP):
                store(sub, xins[sub][:, 0:256].bitcast(mybir.dt.int8))

    if "tr" not in STAGES or "fwd" not in STAGES:
        bogus_stores()
        return

    # ---- forward + quantize, per chunk ----
    qt = pools["qt"].tile([128, 8, GROUP, 128], bf16, tag="qt")
    for p_ in range(8):
        jb = p_ // 2
        par = p_ % 2
        t22 = pt22.tile([128, 512], f32, tag="t22")
        for qi in range(2):
            m = p_ * 2 + qi
            nc.tensor.matmul(
                t22[:, 0:N],
                w1_sb[:, m * 128:(m + 1) * 128],
                V[:, 2 * jb + qi, :, :],
                start=(qi == 0), stop=(qi == 1))
        u = pools["u"].tile([128, N], f32, tag="u")
        eng(B1_ENG).activation(u[:], t22[:, 0:N], Copy, bias=MAGIC, scale=1.0)
        qdst = qt[:, p_, :, 0:TI]
        uv = u[:].rearrange("p (s t) -> p s t", s=GROUP, t=TI)
        sub_v = qv_sb[:, 0:1] if par == 0 else MAGIC
        eng(B2_ENG).tensor_scalar_sub(qdst, uv, sub_v)

    if "inv" not in STAGES:
        bogus_stores()
        return

    # ---- inverse per tile: out[img, pix] block jb accumulates chunks 2jb, 2jb+1
    # half-width PSUM tiles ([128, 512] = 1 bank) for double buffering
    for sub in range(GROUP):
        yout = pools["yout"].tile([TI, 1024], f32, tag="yout")
        for h in range(2):
            outP = pout.tile([128, 512], f32, tag="outP")
            for jbb in range(2):
                jb = 2 * h + jbb
                for qi in range(2):
                    q = 2 * jb + qi
                    nc.tensor.matmul(
                        outP[:, jbb * 256:(jbb + 1) * 256],
                        qt[:, q, sub, :],
                        w2_sb[:, q * 256:(q + 1) * 256],
                        start=(qi == 0), stop=(qi == 1))
            eng(C1_ENG).activation(yout[:, h * 512:(h + 1) * 512], outP[0:TI, :],
                                   Copy, bias=MAGIC, scale=1.0)
        if QUAD:
            eng(C2_ENG).tensor_scalar_sub(yig[:, sub, :], yout[:], MAGIC)
        else:
            yi = pools["yi"].tile([TI, 1024], mybir.dt.int8, tag="yi")
            eng(C2_ENG).tensor_scalar_sub(yi[:], yout[:], MAGIC)
            store(sub, yi[:])
    if QUAD:
        store_quad_flush()


_NC_CACHE = None
PROFILE = False       # test.py sets this to capture an NTFF trace
LAST_RESULT = None    # BassKernelResults of the last run (for exec_time_ns)


def make_in_maps(inputs):
    import ml_dtypes
    x = np.ascontiguousarray(np.asarray(inputs["input"], dtype=np.float32))
    S = x.shape[0]
    assert S == SIZE, f"expected {SIZE} images, got {S}"
    xf = x.reshape(N_CORES, PER_CORE, 1024)

    w = inputs.get("weight")
    wf = inputs.get("weight_factor")
    wfv = float(np.asarray(wf).reshape(-1)[0]) if wf is not None else 65000.0
    if w is not None:
        w = np.asarray(w, dtype=np.float32)
        assert w.shape[0] == 1, "kernel supports n_mult=1"
        w = w[0]
    W1, W2, csub = _build_constants(w, wfv)
    W2 = np.ascontiguousarray(W2.astype(ml_dtypes.bfloat16))
    return [
        {"x": np.ascontiguousarray(xf[c]), "w1": W1, "w2": W2, "qv": csub}
        for c in range(N_CORES)
    ]


def kernel(**inputs) -> np.ndarray:
    global _NC_CACHE, LAST_RESULT
    from concourse.bass_utils import run_bass_kernel_spmd

    in_maps = make_in_maps(inputs)
    if _NC_CACHE is None:
        _NC_CACHE = _build_nc()
    nc = _NC_CACHE
    res = run_bass_kernel_spmd(nc, in_maps, core_ids=list(range(N_CORES)),
                               trace=PROFILE)
    LAST_RESULT = res
    out = np.stack([res.results[c]["y"] for c in range(N_CORES)], axis=0)
    out = out.reshape(1, 1, SIZE, 32, 32).astype(np.float32)
    out = out * 2.0 + 128.0  # device stored round(t2/2)
    return out


if __name__ == "__main__":
    rng = np.random.default_rng(0)
    x = (rng.random((SIZE, 1, 32, 32)) * 255).astype(np.float32)
    y = kernel(input=x)
    print("kernel ran, out shape", y.shape, y.dtype)
